# revision 24
# baseline (speedup 1.0000x reference)
"""AttentionRNN (GRU cell + location-sensitive attention) Trainium2 kernel.

Data-parallel over batch (B=64 -> 8 rows per NeuronCore). All weights are
host-transposed/packed so every matmul operand arrives with the
contraction dim on SBUF partitions, and the whole input set moves in ~12
large DMA transfers split over both HWDGE rings (sync ring: attention
data; scalar ring: GRU/query weights). Everything heavy runs in bf16
(1 cyc/row + fast weight load); accumulation stays fp32 in PSUM.

The PE executes instructions in emission order, so the per-batch loop is
software-pipelined: iteration i issues pa-matmuls for batch i, the
tanh+align group for batch i-2, and the softmax/broadcast/context group
for batch i-3 -- each group's cross-engine producers (ACT/DVE) ran >=1
iteration earlier, so the PE never stalls mid-stream. The GRU + query
projection slot in after pa(1), by which time their weights have landed.

Per core:
  GRU:   psum[8,512]x4 accumulates x@W_ihT / h@W_hhT / bias rows (K=1
         ones trick); sigmoid/tanh gates on ACT+DVE.
  attn:  paT[h,t] psum accumulates annot_wT.T@annT + G_T.T@im2col (G =
         loc_lin_w @ conv_w folded on host); psum -> SBUF bf16, then
         tanh(pa + bias) with the per-(b,h) query bias folded into the
         activation's per-partition bias operand.
  align: v.T @ tanhT (M=1 matmul), softmax along the free dim (logits
         are bounded, so no max-subtraction pass).
  ctx:   (exp row / sum) broadcast to 128 partitions via a K=1 matmul
         against a 1/sum-scaled ones row, then DVE scalar_tensor_tensor
         accumulates new_context.T columns; PE-transposed back to [b,d]
         for one contiguous output DMA.
"""

import sys

sys.path.insert(0, "/opt/trn_rl_repo")

import numpy as np

B, T, H, M = 64, 512, 512, 80
LOC_DIM, LOC_K, LOC_PAD = 32, 31, 15
NCORES = 8
BL = B // NCORES  # 8 batch rows per core
KX = M + H  # 592 = GRU input features [memory | context]
N_WARM = 56  # PE warmup matmuls (warm the clock gate before real work)

# packed bf16 tile columns (sb0: attention-critical, sb1: GRU smalls)
_B_ONE = 0  # ones row [1,128]
_B_V = 128  # v columns (4)
_B_GT = 132  # gt rows 0:62, 512 cols
_B_IM2 = 644  # im2col rows 0:62, 8*512 cols
_SB0_COLS = 644 + 4096
_B_XT = 0  # xT 5 k-tiles x 8
_B_HT = 40  # hT 4 k-tiles x 8
_B_ON8 = 72  # ones row [1,8]
_B_BIH = 80  # b_ih row [1,1536]
_B_BHH = 1616  # b_hh row [1,1536]
_SB1_COLS = 3152

# packed f32 tile columns
_AB0 = 0  # ab k-tiles (4)
_HN0 = 4  # hnat rows 0:8, 512 cols
_ID80 = 516  # 8x8 identity
_IDF0 = 524  # 128x128 identity
_SF_COLS = 652

_CACHE = {}


def _build():
    import concourse.bass as bass
    import concourse.tile as tile
    from concourse import bacc, mybir

    f32 = mybir.dt.float32
    bf16 = mybir.dt.bfloat16
    AF = mybir.ActivationFunctionType
    ALU = mybir.AluOpType

    nc = bacc.Bacc()

    d_ann = nc.dram_tensor("annp", [128, BL, 4, T], bf16, kind="ExternalInput")
    d_w1 = nc.dram_tensor("w1p", [128, 5 * 3 * H], bf16, kind="ExternalInput")
    d_w2 = nc.dram_tensor("w2p", [128, 4 * 3 * H], bf16, kind="ExternalInput")
    d_w3 = nc.dram_tensor("w3p", [128, 4 * H], bf16, kind="ExternalInput")
    d_aw = nc.dram_tensor("awp", [128, 4 * H], bf16, kind="ExternalInput")
    d_sf = nc.dram_tensor("sfp", [128, _SF_COLS], f32, kind="ExternalInput")
    d_sb0 = nc.dram_tensor("sb0p", [128, _SB0_COLS], bf16, kind="ExternalInput")
    d_sb1 = nc.dram_tensor("sb1p", [128, _SB1_COLS], bf16, kind="ExternalInput")

    d_rnn = nc.dram_tensor("rnn_out", [BL, H], f32, kind="ExternalOutput")
    d_ctx = nc.dram_tensor("ctx_out", [BL, H], f32, kind="ExternalOutput")
    d_al = nc.dram_tensor("align_out", [BL, T], f32, kind="ExternalOutput")

    with tile.TileContext(nc) as tc:
        with (
            tc.tile_pool(name="wpool", bufs=1) as wp,
            tc.tile_pool(name="gwork", bufs=1) as gwork,
            tc.tile_pool(name="work", bufs=3) as work,
            tc.tile_pool(name="parawp", bufs=32) as parawp,
            tc.tile_pool(name="tanhp", bufs=10) as tanhp,
            tc.tile_pool(name="small", bufs=1) as small,
            tc.tile_pool(name="ps_gru", bufs=2, space="PSUM") as ps_gru,
            tc.tile_pool(name="ps_pa", bufs=2, space="PSUM") as ps_pa,
            tc.tile_pool(name="ps_bc", bufs=2, space="PSUM") as ps_bc,
            tc.tile_pool(name="ps_sm", bufs=2, space="PSUM") as ps_sm,
        ):
            # ---- DMA: pa-critical data leads BOTH rings; GRU weights
            #      follow on the scalar ring; later ann chunks trail ----
            sb0 = wp.tile([128, _SB0_COLS], bf16, tag="sb0")
            nc.scalar.dma_start(sb0[:], d_sb0[:])
            ann = wp.tile([128, BL, 4, T], bf16, tag="ann")
            nc.sync.dma_start(ann[:, 0:1], d_ann[:, 0:1])
            awt = wp.tile([128, 4 * H], bf16, tag="awt")
            nc.sync.dma_start(awt[:], d_aw[:])
            sb1 = wp.tile([128, _SB1_COLS], bf16, tag="sb1")
            nc.scalar.dma_start(sb1[:], d_sb1[:])
            nc.sync.dma_start(ann[:, 1:3], d_ann[:, 1:3])
            w1 = wp.tile([128, 5 * 3 * H], bf16, tag="w1")
            nc.scalar.dma_start(w1[:], d_w1[:])
            nc.sync.dma_start(ann[:, 3:5], d_ann[:, 3:5])
            w2 = wp.tile([128, 4 * 3 * H], bf16, tag="w2")
            nc.scalar.dma_start(w2[:], d_w2[:])
            nc.sync.dma_start(ann[:, 5:8], d_ann[:, 5:8])
            sf = wp.tile([128, _SF_COLS], f32, tag="sf")
            nc.scalar.dma_start(sf[:], d_sf[:])
            w3 = wp.tile([128, 4 * H], bf16, tag="w3")
            nc.scalar.dma_start(w3[:], d_w3[:])

            # ---- slices ----
            onesb = sb0[0:1, _B_ONE : _B_ONE + 128]
            v = [sb0[:, _B_V + i : _B_V + i + 1] for i in range(4)]
            gt = sb0[0:62, _B_GT : _B_GT + 512]
            im2 = sb0[0:62, _B_IM2 : _B_IM2 + BL * T]
            xt = [
                sb1[0 : min(128, KX - 128 * i), _B_XT + 8 * i : _B_XT + 8 * (i + 1)]
                for i in range(5)
            ]
            ht = [sb1[:, _B_HT + 8 * i : _B_HT + 8 * (i + 1)] for i in range(4)]
            ones8 = sb1[0:1, _B_ON8 : _B_ON8 + BL]
            bih = sb1[0:1, _B_BIH : _B_BIH + 1536]
            bhh = sb1[0:1, _B_BHH : _B_BHH + 1536]
            ab = [sf[:, _AB0 + i : _AB0 + i + 1] for i in range(4)]
            hnat = sf[0:BL, _HN0 : _HN0 + 512]
            id8 = sf[0:BL, _ID80 : _ID80 + 8]
            idf = sf[:, _IDF0 : _IDF0 + 128]
            wih = [
                w1[0 : min(128, KX - 128 * i), 1536 * i : 1536 * (i + 1)]
                for i in range(5)
            ]
            whh = [w2[:, 1536 * i : 1536 * (i + 1)] for i in range(4)]
            qw = [w3[:, 512 * i : 512 * (i + 1)] for i in range(4)]
            aw = [awt[:, 512 * i : 512 * (i + 1)] for i in range(4)]

            # ---- PE warmup on a memset tile (no DMA dependency) ----
            wtile = wp.tile([128, 128], bf16, tag="wtile")
            nc.vector.memset(wtile[:], 0.25)
            warm = ps_sm.tile([128, 128], f32, tag="sm", name="warm_ps")
            for _ in range(N_WARM):
                nc.tensor.matmul(warm[:], wtile[:], wtile[:], start=True, stop=True)

            pa_raw = {}

            def emit_pa(b):
                for m_ in range(4):
                    pp = ps_pa.tile([128, T], f32, tag="pa")
                    nc.tensor.matmul(
                        pp[:],
                        gt[:, 128 * m_ : 128 * (m_ + 1)],
                        im2[:, T * b : T * (b + 1)],
                        start=True,
                        stop=False,
                    )
                    for k in range(4):
                        nc.tensor.matmul(
                            pp[:],
                            aw[k][:, 128 * m_ : 128 * (m_ + 1)],
                            ann[:, b, k, :],
                            start=False,
                            stop=(k == 3),
                        )
                    raw = parawp.tile(
                        [128, T], bf16, tag="paraw", name=f"paraw{b}_{m_}"
                    )
                    nc.vector.tensor_copy(raw[:], pp[:])
                    pa_raw[(b, m_)] = raw

            def emit_gru():
                def gru_chunk(name, g0, use_x, use_h):
                    pgc = ps_gru.tile([BL, 512], f32, tag="pg", name=name)
                    mms = []
                    if use_x:
                        for i in range(5):
                            mms.append((xt[i], wih[i][:, g0 : g0 + 512]))
                        mms.append((ones8, bih[:, g0 : g0 + 512]))
                    if use_h:
                        for i in range(4):
                            mms.append((ht[i], whh[i][:, g0 : g0 + 512]))
                        mms.append((ones8, bhh[:, g0 : g0 + 512]))
                    for j, (l, r_) in enumerate(mms):
                        nc.tensor.matmul(
                            pgc[:], l, r_, start=(j == 0), stop=(j == len(mms) - 1)
                        )
                    return pgc

                pg_r = gru_chunk("pg_r", 0, True, True)
                r_sb = gwork.tile([BL, H], f32, tag="r_sb")
                nc.scalar.activation(r_sb[:], pg_r[:], AF.Sigmoid)
                pg_z = gru_chunk("pg_z", 512, True, True)
                z_sb = gwork.tile([BL, H], f32, tag="z_sb")
                nc.scalar.activation(z_sb[:], pg_z[:], AF.Sigmoid)
                pg_in = gru_chunk("pg_in", 1024, True, False)
                pg_hn = gru_chunk("pg_hn", 1024, False, True)
                t1 = gwork.tile([BL, H], f32, tag="t1")
                nc.vector.tensor_mul(t1[:], r_sb[:], pg_hn[:])
                t2 = gwork.tile([BL, H], f32, tag="t2")
                nc.vector.tensor_add(t2[:], t1[:], pg_in[:])
                n_ = gwork.tile([BL, H], f32, tag="n")
                nc.scalar.activation(n_[:], t2[:], AF.Tanh)
                t3 = gwork.tile([BL, H], f32, tag="t3")
                nc.vector.tensor_sub(t3[:], hnat, n_[:])
                t4 = gwork.tile([BL, H], f32, tag="t4")
                nc.vector.tensor_mul(t4[:], z_sb[:], t3[:])
                rnn = gwork.tile([BL, H], f32, tag="rnn")
                nc.vector.tensor_add(rnn[:], n_[:], t4[:])
                nc.gpsimd.dma_start(d_rnn[:], rnn[:])

                biasT = []
                for m_ in range(4):
                    pt = ps_sm.tile([128, BL], f32, tag="sm", name="ptr")
                    nc.tensor.transpose(
                        pt[:], rnn[:, 128 * m_ : 128 * (m_ + 1)], id8
                    )
                    st = gwork.tile([128, BL], bf16, tag=f"rnnT{m_}")
                    nc.vector.tensor_copy(st[:], pt[:])
                    biasT.append(st)
                rnnT = biasT[:]
                biasT = []
                for m_ in range(4):
                    pq = ps_sm.tile([128, BL], f32, tag="sm", name="pq")
                    for k in range(4):
                        nc.tensor.matmul(
                            pq[:],
                            qw[k][:, 128 * m_ : 128 * (m_ + 1)],
                            rnnT[k][:],
                            start=(k == 0),
                            stop=(k == 3),
                        )
                    bt = gwork.tile([128, BL], f32, tag=f"biasT{m_}")
                    nc.vector.tensor_scalar_add(bt[:], pq[:], ab[m_])
                    biasT.append(bt)
                return biasT

            nctxT = [
                small.tile([128, BL], f32, tag=f"nctxT{m_}", name=f"nctxT{m_}")
                for m_ in range(4)
            ]
            scr2 = [
                small.tile([128, T], bf16, tag="scr_d", name="scr_d"),
                small.tile([128, T], bf16, tag="scr_g", name="scr_g"),
            ]
            tanh_of = {}
            pal_of = {}

            def emit_tanh_align(b, biasT):
                tt = []
                for m_ in range(4):
                    th = tanhp.tile([128, T], bf16, tag="tanh")
                    nc.scalar.activation(
                        th[:],
                        pa_raw.pop((b, m_))[:],
                        AF.Tanh,
                        bias=biasT[m_][:, b : b + 1],
                    )
                    tt.append(th)
                tanh_of[b] = tt
                pal = ps_sm.tile([1, T], f32, tag="sm", name="pal")
                for m_ in range(4):
                    nc.tensor.matmul(
                        pal[:], v[m_], tt[m_][:], start=(m_ == 0), stop=(m_ == 3)
                    )
                pal_of[b] = pal

            def emit_ctx(b):
                pal = pal_of.pop(b)
                tanh_of.pop(b, None)
                sm = work.tile([1, 1], f32, tag="sm_")
                exw = work.tile([1, T], bf16, tag="exw")
                nc.scalar.activation(exw[:], pal[:], AF.Exp, accum_out=sm[:])
                rcp = work.tile([1, 1], f32, tag="rcp")
                nc.vector.reciprocal(rcp[:], sm[:])
                rball = work.tile([1, 128], bf16, tag="rball")
                nc.vector.tensor_scalar_mul(rball[:], onesb, rcp[:])
                alw = work.tile([1, T], f32, tag="alw")
                nc.vector.tensor_scalar_mul(alw[:], exw[:], rcp[:])
                nc.gpsimd.dma_start(d_al[b : b + 1, :], alw[:])
                pbc = ps_bc.tile([128, T], f32, tag="bc")
                nc.tensor.matmul(pbc[:], rball[:], exw[:], start=True, stop=True)
                for m_ in range(4):
                    nc.vector.scalar_tensor_tensor(
                        out=scr2[m_ % 2][:],
                        in0=ann[:, b, m_, :],
                        scalar=1.0,
                        in1=pbc[:],
                        op0=ALU.mult,
                        op1=ALU.mult,
                        accum_out=nctxT[m_][:, b : b + 1],
                    )

            # ---- software-pipelined schedule ----
            biasT = None
            for i in range(11):
                if i < 8:
                    emit_pa(i)
                if i == 1:
                    biasT = emit_gru()
                b1 = i - 2
                if 0 <= b1 < 8:
                    emit_tanh_align(b1, biasT)
                b2 = i - 3
                if 0 <= b2 < 8:
                    emit_ctx(b2)

            # new_context.T -> [b, d] via PE transpose, one output DMA
            ctx_sb = small.tile([BL, H], f32, tag="ctx_sb")
            for m_ in range(4):
                pt = ps_sm.tile([BL, 128], f32, tag="sm", name="ctxtr")
                nc.tensor.transpose(pt[:], nctxT[m_][:], idf)
                nc.vector.tensor_copy(ctx_sb[:, 128 * m_ : 128 * (m_ + 1)], pt[:])
            nc.gpsimd.dma_start(d_ctx[:], ctx_sb[:])

    nc.finalize()
    return nc


def _get_nc():
    if "nc" not in _CACHE:
        _CACHE["nc"] = _build()
    return _CACHE["nc"]


def _host_prep(inputs):
    """Build per-core input maps (host-side layout/packing prep only)."""
    import ml_dtypes

    bf = ml_dtypes.bfloat16

    memory = np.asarray(inputs["memory"], np.float32)
    context = np.asarray(inputs["context"], np.float32)
    rnn_state = np.asarray(inputs["rnn_state"], np.float32)
    annotations = np.asarray(inputs["annotations"], np.float32)
    attention_vec = np.asarray(inputs["attention_vec"], np.float32)
    W_ih = np.asarray(inputs["W_ih"], np.float32)
    W_hh = np.asarray(inputs["W_hh"], np.float32)
    b_ih = np.asarray(inputs["b_ih"], np.float32)
    b_hh = np.asarray(inputs["b_hh"], np.float32)
    loc_conv_w = np.asarray(inputs["loc_conv_w"], np.float32)
    loc_lin_w = np.asarray(inputs["loc_lin_w"], np.float32)
    loc_lin_b = np.asarray(inputs["loc_lin_b"], np.float32)
    query_w = np.asarray(inputs["query_w"], np.float32)
    query_b = np.asarray(inputs["query_b"], np.float32)
    annot_w = np.asarray(inputs["annot_w"], np.float32)
    annot_b = np.asarray(inputs["annot_b"], np.float32)
    v_w = np.asarray(inputs["v_w"], np.float32)

    wihT = np.zeros((640, 3 * H), np.float32)
    wihT[:KX] = W_ih.T
    w1p = np.ascontiguousarray(
        wihT.reshape(5, 128, 3 * H).transpose(1, 0, 2).reshape(128, 5 * 3 * H)
    ).astype(bf)
    whhT = np.ascontiguousarray(W_hh.T)
    w2p = np.ascontiguousarray(
        whhT.reshape(4, 128, 3 * H).transpose(1, 0, 2).reshape(128, 4 * 3 * H)
    ).astype(bf)
    qwT = np.ascontiguousarray(query_w.T)
    w3p = np.ascontiguousarray(
        qwT.reshape(4, 128, H).transpose(1, 0, 2).reshape(128, 4 * H)
    ).astype(bf)
    awT = np.ascontiguousarray(annot_w.T)
    awp = np.ascontiguousarray(
        awT.reshape(4, 128, H).transpose(1, 0, 2).reshape(128, 4 * H)
    ).astype(bf)

    g = loc_lin_w @ loc_conv_w.reshape(LOC_DIM, 2 * LOC_K)  # (512, 62)
    gT = np.ascontiguousarray(g.T)
    ab_col = (query_b + annot_b + loc_lin_b).reshape(4, 128).T  # (128, 4)
    v_col = v_w.reshape(4, 128).T.astype(bf)

    av_pad = np.zeros((B, 2, T + 2 * LOC_PAD), np.float32)
    av_pad[:, :, LOC_PAD : LOC_PAD + T] = attention_vec
    im2col = np.empty((B, 2, LOC_K, T), np.float32)
    for k in range(LOC_K):
        im2col[:, :, k, :] = av_pad[:, :, k : k + T]
    im2col = im2col.reshape(B, 2 * LOC_K, T)

    x = np.concatenate([memory, context], axis=1)
    xpad = np.zeros((B, 640), np.float32)
    xpad[:, :KX] = x

    annT = annotations.transpose(0, 2, 1).astype(bf)  # (B, H, T)
    annp_all = np.ascontiguousarray(
        annT.reshape(B // BL, BL, 4, 128, T).transpose(0, 3, 1, 2, 4)
    )  # (ncores, 128, BL, 4, T)

    in_maps = []
    for c in range(NCORES):
        sl = slice(c * BL, (c + 1) * BL)

        sb0p = np.zeros((128, _SB0_COLS), bf)
        sb0p[0:1, _B_ONE : _B_ONE + 128] = bf(1.0)
        sb0p[:, _B_V : _B_V + 4] = v_col
        sb0p[0:62, _B_GT : _B_GT + 512] = gT.astype(bf)
        sb1p = np.zeros((128, _SB1_COLS), bf)
        sb1p[:, _B_XT : _B_XT + 40] = (
            xpad[sl].reshape(BL, 5, 128).transpose(2, 1, 0).reshape(128, 40)
        ).astype(bf)
        sb1p[:, _B_HT : _B_HT + 32] = (
            rnn_state[sl].reshape(BL, 4, 128).transpose(2, 1, 0).reshape(128, 32)
        ).astype(bf)
        sb1p[0:1, _B_ON8 : _B_ON8 + BL] = bf(1.0)
        sb1p[0:1, _B_BIH : _B_BIH + 1536] = b_ih.astype(bf)
        sb1p[0:1, _B_BHH : _B_BHH + 1536] = b_hh.astype(bf)

        sfp = np.zeros((128, _SF_COLS), np.float32)
        sfp[:, _AB0 : _AB0 + 4] = ab_col
        sfp[0:BL, _HN0 : _HN0 + 512] = rnn_state[sl]
        sfp[0:BL, _ID80 : _ID80 + 8] = np.eye(8)
        sfp[:, _IDF0 : _IDF0 + 128] = np.eye(128)

        sb0p[0:62, _B_IM2 : _B_IM2 + BL * T] = (
            im2col[sl].transpose(1, 0, 2).reshape(62, BL * T).astype(bf)
        )

        in_maps.append(
            dict(
                annp=annp_all[c],
                w1p=w1p,
                w2p=w2p,
                w3p=w3p,
                awp=awp,
                sfp=sfp,
                sb0p=sb0p,
                sb1p=sb1p,
            )
        )
    return in_maps


def run(inputs, trace=False, **kw):
    from concourse.bass_utils import run_bass_kernel_spmd

    nc = _get_nc()
    in_maps = _host_prep(inputs)
    res = run_bass_kernel_spmd(nc, in_maps, list(range(NCORES)), trace=trace, **kw)
    rnn = np.concatenate([res.results[c]["rnn_out"] for c in range(NCORES)], axis=0)
    ctx = np.concatenate([res.results[c]["ctx_out"] for c in range(NCORES)], axis=0)
    al = np.concatenate([res.results[c]["align_out"] for c in range(NCORES)], axis=0)
    return (rnn, ctx, al), res


def kernel(**inputs):
    (rnn, ctx, al), _ = run(inputs)
    return (rnn, ctx, al)


# revision 25
# speedup vs baseline: 1.0368x; 1.0368x over previous
"""AttentionRNN (GRU cell + location-sensitive attention) Trainium2 kernel.

Data-parallel over batch (B=64 -> 8 rows per NeuronCore). All weights are
host-transposed/packed so every matmul operand arrives with the
contraction dim on SBUF partitions, and the whole input set moves in ~12
large DMA transfers split over both HWDGE rings (sync ring: attention
data; scalar ring: GRU/query weights). Everything heavy runs in bf16
(1 cyc/row + fast weight load); accumulation stays fp32 in PSUM.

The PE executes instructions in emission order, so the per-batch loop is
software-pipelined: iteration i issues pa-matmuls for batch i, the
tanh+align group for batch i-2, and the softmax/broadcast/context group
for batch i-3 -- each group's cross-engine producers (ACT/DVE) ran >=1
iteration earlier, so the PE never stalls mid-stream. The GRU + query
projection slot in after pa(1), by which time their weights have landed.

Per core:
  GRU:   psum[8,512]x4 accumulates x@W_ihT / h@W_hhT / bias rows (K=1
         ones trick); sigmoid/tanh gates on ACT+DVE.
  attn:  paT[h,t] psum accumulates annot_wT.T@annT + G_T.T@im2col (G =
         loc_lin_w @ conv_w folded on host); psum -> SBUF bf16, then
         tanh(pa + bias) with the per-(b,h) query bias folded into the
         activation's per-partition bias operand.
  align: v.T @ tanhT (M=1 matmul), softmax along the free dim (logits
         are bounded, so no max-subtraction pass).
  ctx:   (exp row / sum) broadcast to 128 partitions via a K=1 matmul
         against a 1/sum-scaled ones row, then DVE scalar_tensor_tensor
         accumulates new_context.T columns; PE-transposed back to [b,d]
         for one contiguous output DMA.
"""

import sys

sys.path.insert(0, "/opt/trn_rl_repo")

import numpy as np

B, T, H, M = 64, 512, 512, 80
LOC_DIM, LOC_K, LOC_PAD = 32, 31, 15
NCORES = 8
BL = B // NCORES  # 8 batch rows per core
KX = M + H  # 592 = GRU input features [memory | context]
N_WARM = 80  # PE warmup matmuls (warm the clock gate before real work)

# packed bf16 tile columns (sb0: attention-critical, sb1: GRU smalls)
_B_ONE = 0  # ones row [1,128]
_B_V = 128  # v columns (4)
_B_GT = 132  # gt rows 0:62, 512 cols
_B_IM2 = 644  # im2col rows 0:62, 8*512 cols
_SB0_COLS = 644 + 4096
_B_XT = 0  # xT 5 k-tiles x 8
_B_HT = 40  # hT 4 k-tiles x 8
_B_ON8 = 72  # ones row [1,8]
_B_BIH = 80  # b_ih row [1,1536]
_B_BHH = 1616  # b_hh row [1,1536]
_SB1_COLS = 3152

# packed f32 tile columns
_AB0 = 0  # ab k-tiles (4)
_HN0 = 4  # hnat rows 0:8, 512 cols
_ID80 = 516  # 8x8 identity
_IDF0 = 524  # 128x128 identity
_SF_COLS = 652

_CACHE = {}


def _build():
    import concourse.bass as bass
    import concourse.tile as tile
    from concourse import bacc, mybir

    f32 = mybir.dt.float32
    bf16 = mybir.dt.bfloat16
    AF = mybir.ActivationFunctionType
    ALU = mybir.AluOpType

    nc = bacc.Bacc()

    d_ann = nc.dram_tensor("annp", [128, BL, 4, T], bf16, kind="ExternalInput")
    d_w1 = nc.dram_tensor("w1p", [128, 5 * 3 * H], bf16, kind="ExternalInput")
    d_w2 = nc.dram_tensor("w2p", [128, 4 * 3 * H], bf16, kind="ExternalInput")
    d_w3 = nc.dram_tensor("w3p", [128, 4 * H], bf16, kind="ExternalInput")
    d_aw = nc.dram_tensor("awp", [128, 4 * H], bf16, kind="ExternalInput")
    d_sf = nc.dram_tensor("sfp", [128, _SF_COLS], f32, kind="ExternalInput")
    d_sb0 = nc.dram_tensor("sb0p", [128, _SB0_COLS], bf16, kind="ExternalInput")
    d_sb1 = nc.dram_tensor("sb1p", [128, _SB1_COLS], bf16, kind="ExternalInput")

    d_rnn = nc.dram_tensor("rnn_out", [BL, H], f32, kind="ExternalOutput")
    d_ctx = nc.dram_tensor("ctx_out", [BL, H], f32, kind="ExternalOutput")
    d_al = nc.dram_tensor("align_out", [BL, T], f32, kind="ExternalOutput")

    with tile.TileContext(nc) as tc:
        with (
            tc.tile_pool(name="wpool", bufs=1) as wp,
            tc.tile_pool(name="gwork", bufs=1) as gwork,
            tc.tile_pool(name="work", bufs=3) as work,
            tc.tile_pool(name="parawp", bufs=32) as parawp,
            tc.tile_pool(name="tanhp", bufs=10) as tanhp,
            tc.tile_pool(name="small", bufs=1) as small,
            tc.tile_pool(name="ps_gru", bufs=2, space="PSUM") as ps_gru,
            tc.tile_pool(name="ps_pa", bufs=2, space="PSUM") as ps_pa,
            tc.tile_pool(name="ps_bc", bufs=2, space="PSUM") as ps_bc,
            tc.tile_pool(name="ps_sm", bufs=2, space="PSUM") as ps_sm,
        ):
            # ---- DMA: pa-critical data leads BOTH rings; GRU weights
            #      follow on the scalar ring; later ann chunks trail ----
            sb0 = wp.tile([128, _SB0_COLS], bf16, tag="sb0")
            nc.scalar.dma_start(sb0[:], d_sb0[:])
            ann = wp.tile([128, BL, 4, T], bf16, tag="ann")
            nc.sync.dma_start(ann[:, 0:1], d_ann[:, 0:1])
            awt = wp.tile([128, 4 * H], bf16, tag="awt")
            nc.sync.dma_start(awt[:], d_aw[:])
            sb1 = wp.tile([128, _SB1_COLS], bf16, tag="sb1")
            nc.scalar.dma_start(sb1[:], d_sb1[:])
            nc.sync.dma_start(ann[:, 1:3], d_ann[:, 1:3])
            w1 = wp.tile([128, 5 * 3 * H], bf16, tag="w1")
            nc.scalar.dma_start(w1[:], d_w1[:])
            nc.sync.dma_start(ann[:, 3:5], d_ann[:, 3:5])
            w2 = wp.tile([128, 4 * 3 * H], bf16, tag="w2")
            nc.scalar.dma_start(w2[:], d_w2[:])
            nc.sync.dma_start(ann[:, 5:8], d_ann[:, 5:8])
            sf = wp.tile([128, _SF_COLS], f32, tag="sf")
            nc.scalar.dma_start(sf[:], d_sf[:])
            w3 = wp.tile([128, 4 * H], bf16, tag="w3")
            nc.scalar.dma_start(w3[:], d_w3[:])

            # ---- slices ----
            onesb = sb0[0:1, _B_ONE : _B_ONE + 128]
            v = [sb0[:, _B_V + i : _B_V + i + 1] for i in range(4)]
            gt = sb0[0:62, _B_GT : _B_GT + 512]
            im2 = sb0[0:62, _B_IM2 : _B_IM2 + BL * T]
            xt = [
                sb1[0 : min(128, KX - 128 * i), _B_XT + 8 * i : _B_XT + 8 * (i + 1)]
                for i in range(5)
            ]
            ht = [sb1[:, _B_HT + 8 * i : _B_HT + 8 * (i + 1)] for i in range(4)]
            ones8 = sb1[0:1, _B_ON8 : _B_ON8 + BL]
            bih = sb1[0:1, _B_BIH : _B_BIH + 1536]
            bhh = sb1[0:1, _B_BHH : _B_BHH + 1536]
            ab = [sf[:, _AB0 + i : _AB0 + i + 1] for i in range(4)]
            hnat = sf[0:BL, _HN0 : _HN0 + 512]
            id8 = sf[0:BL, _ID80 : _ID80 + 8]
            idf = sf[:, _IDF0 : _IDF0 + 128]
            wih = [
                w1[0 : min(128, KX - 128 * i), 1536 * i : 1536 * (i + 1)]
                for i in range(5)
            ]
            whh = [w2[:, 1536 * i : 1536 * (i + 1)] for i in range(4)]
            qw = [w3[:, 512 * i : 512 * (i + 1)] for i in range(4)]
            aw = [awt[:, 512 * i : 512 * (i + 1)] for i in range(4)]

            # ---- PE warmup on a memset tile (no DMA dependency) ----
            wtile = wp.tile([128, 128], bf16, tag="wtile")
            nc.vector.memset(wtile[:], 0.25)
            warm = ps_sm.tile([128, 128], f32, tag="sm", name="warm_ps")
            for _ in range(N_WARM):
                nc.tensor.matmul(warm[:], wtile[:], wtile[:], start=True, stop=True)

            pa_raw = {}

            def emit_pa(b):
                for m_ in range(4):
                    pp = ps_pa.tile([128, T], f32, tag="pa")
                    nc.tensor.matmul(
                        pp[:],
                        gt[:, 128 * m_ : 128 * (m_ + 1)],
                        im2[:, T * b : T * (b + 1)],
                        start=True,
                        stop=False,
                    )
                    for k in range(4):
                        nc.tensor.matmul(
                            pp[:],
                            aw[k][:, 128 * m_ : 128 * (m_ + 1)],
                            ann[:, b, k, :],
                            start=False,
                            stop=(k == 3),
                        )
                    raw = parawp.tile(
                        [128, T], bf16, tag="paraw", name=f"paraw{b}_{m_}"
                    )
                    nc.vector.tensor_copy(raw[:], pp[:])
                    pa_raw[(b, m_)] = raw

            def emit_gru():
                def gru_chunk(name, g0, use_x, use_h):
                    pgc = ps_gru.tile([BL, 512], f32, tag="pg", name=name)
                    mms = []
                    if use_x:
                        for i in range(5):
                            mms.append((xt[i], wih[i][:, g0 : g0 + 512]))
                        mms.append((ones8, bih[:, g0 : g0 + 512]))
                    if use_h:
                        for i in range(4):
                            mms.append((ht[i], whh[i][:, g0 : g0 + 512]))
                        mms.append((ones8, bhh[:, g0 : g0 + 512]))
                    for j, (l, r_) in enumerate(mms):
                        nc.tensor.matmul(
                            pgc[:], l, r_, start=(j == 0), stop=(j == len(mms) - 1)
                        )
                    return pgc

                pg_r = gru_chunk("pg_r", 0, True, True)
                r_sb = gwork.tile([BL, H], f32, tag="r_sb")
                nc.scalar.activation(r_sb[:], pg_r[:], AF.Sigmoid)
                pg_z = gru_chunk("pg_z", 512, True, True)
                z_sb = gwork.tile([BL, H], f32, tag="z_sb")
                nc.scalar.activation(z_sb[:], pg_z[:], AF.Sigmoid)
                pg_in = gru_chunk("pg_in", 1024, True, False)
                pg_hn = gru_chunk("pg_hn", 1024, False, True)
                t1 = gwork.tile([BL, H], f32, tag="t1")
                nc.vector.tensor_mul(t1[:], r_sb[:], pg_hn[:])
                t2 = gwork.tile([BL, H], f32, tag="t2")
                nc.vector.tensor_add(t2[:], t1[:], pg_in[:])
                n_ = gwork.tile([BL, H], f32, tag="n")
                nc.scalar.activation(n_[:], t2[:], AF.Tanh)
                t3 = gwork.tile([BL, H], f32, tag="t3")
                nc.vector.tensor_sub(t3[:], hnat, n_[:])
                t4 = gwork.tile([BL, H], f32, tag="t4")
                nc.vector.tensor_mul(t4[:], z_sb[:], t3[:])
                rnn = gwork.tile([BL, H], f32, tag="rnn")
                nc.vector.tensor_add(rnn[:], n_[:], t4[:])
                nc.gpsimd.dma_start(d_rnn[:], rnn[:])

                biasT = []
                for m_ in range(4):
                    pt = ps_sm.tile([128, BL], f32, tag="sm", name="ptr")
                    nc.tensor.transpose(
                        pt[:], rnn[:, 128 * m_ : 128 * (m_ + 1)], id8
                    )
                    st = gwork.tile([128, BL], bf16, tag=f"rnnT{m_}")
                    nc.vector.tensor_copy(st[:], pt[:])
                    biasT.append(st)
                rnnT = biasT[:]
                biasT = []
                for m_ in range(4):
                    pq = ps_sm.tile([128, BL], f32, tag="sm", name="pq")
                    for k in range(4):
                        nc.tensor.matmul(
                            pq[:],
                            qw[k][:, 128 * m_ : 128 * (m_ + 1)],
                            rnnT[k][:],
                            start=(k == 0),
                            stop=(k == 3),
                        )
                    bt = gwork.tile([128, BL], f32, tag=f"biasT{m_}")
                    nc.vector.tensor_scalar_add(bt[:], pq[:], ab[m_])
                    biasT.append(bt)
                return biasT

            nctxT = [
                small.tile([128, BL], f32, tag=f"nctxT{m_}", name=f"nctxT{m_}")
                for m_ in range(4)
            ]
            scr2 = [
                small.tile([128, T], bf16, tag="scr_d", name="scr_d"),
                small.tile([128, T], bf16, tag="scr_g", name="scr_g"),
            ]
            tanh_of = {}
            pal_of = {}

            def emit_tanh_align(b, biasT):
                tt = []
                for m_ in range(4):
                    th = tanhp.tile([128, T], bf16, tag="tanh")
                    nc.scalar.activation(
                        th[:],
                        pa_raw.pop((b, m_))[:],
                        AF.Tanh,
                        bias=biasT[m_][:, b : b + 1],
                    )
                    tt.append(th)
                tanh_of[b] = tt
                pal = ps_sm.tile([1, T], f32, tag="sm", name="pal")
                for m_ in range(4):
                    nc.tensor.matmul(
                        pal[:], v[m_], tt[m_][:], start=(m_ == 0), stop=(m_ == 3)
                    )
                pal_of[b] = pal

            def emit_ctx(b):
                pal = pal_of.pop(b)
                tanh_of.pop(b, None)
                sm = work.tile([1, 1], f32, tag="sm_")
                exw = work.tile([1, T], bf16, tag="exw")
                nc.scalar.activation(exw[:], pal[:], AF.Exp, accum_out=sm[:])
                rcp = work.tile([1, 1], f32, tag="rcp")
                nc.vector.reciprocal(rcp[:], sm[:])
                rball = work.tile([1, 128], bf16, tag="rball")
                nc.vector.tensor_scalar_mul(rball[:], onesb, rcp[:])
                alw = work.tile([1, T], f32, tag="alw")
                nc.vector.tensor_scalar_mul(alw[:], exw[:], rcp[:])
                nc.gpsimd.dma_start(d_al[b : b + 1, :], alw[:])
                pbc = ps_bc.tile([128, T], f32, tag="bc")
                nc.tensor.matmul(pbc[:], rball[:], exw[:], start=True, stop=True)
                for m_ in range(4):
                    nc.vector.scalar_tensor_tensor(
                        out=scr2[m_ % 2][:],
                        in0=ann[:, b, m_, :],
                        scalar=1.0,
                        in1=pbc[:],
                        op0=ALU.mult,
                        op1=ALU.mult,
                        accum_out=nctxT[m_][:, b : b + 1],
                    )

            # ---- software-pipelined schedule ----
            biasT = None
            for i in range(11):
                if i < 8:
                    emit_pa(i)
                if i == 1:
                    biasT = emit_gru()
                b1 = i - 2
                if 0 <= b1 < 8:
                    emit_tanh_align(b1, biasT)
                b2 = i - 3
                if 0 <= b2 < 8:
                    emit_ctx(b2)

            # new_context.T -> [b, d] via PE transpose, one output DMA
            ctx_sb = small.tile([BL, H], f32, tag="ctx_sb")
            for m_ in range(4):
                pt = ps_sm.tile([BL, 128], f32, tag="sm", name="ctxtr")
                nc.tensor.transpose(pt[:], nctxT[m_][:], idf)
                nc.vector.tensor_copy(ctx_sb[:, 128 * m_ : 128 * (m_ + 1)], pt[:])
            nc.gpsimd.dma_start(d_ctx[:], ctx_sb[:])

    nc.finalize()
    return nc


def _get_nc():
    if "nc" not in _CACHE:
        _CACHE["nc"] = _build()
    return _CACHE["nc"]


def _host_prep(inputs):
    """Build per-core input maps (host-side layout/packing prep only)."""
    import ml_dtypes

    bf = ml_dtypes.bfloat16

    memory = np.asarray(inputs["memory"], np.float32)
    context = np.asarray(inputs["context"], np.float32)
    rnn_state = np.asarray(inputs["rnn_state"], np.float32)
    annotations = np.asarray(inputs["annotations"], np.float32)
    attention_vec = np.asarray(inputs["attention_vec"], np.float32)
    W_ih = np.asarray(inputs["W_ih"], np.float32)
    W_hh = np.asarray(inputs["W_hh"], np.float32)
    b_ih = np.asarray(inputs["b_ih"], np.float32)
    b_hh = np.asarray(inputs["b_hh"], np.float32)
    loc_conv_w = np.asarray(inputs["loc_conv_w"], np.float32)
    loc_lin_w = np.asarray(inputs["loc_lin_w"], np.float32)
    loc_lin_b = np.asarray(inputs["loc_lin_b"], np.float32)
    query_w = np.asarray(inputs["query_w"], np.float32)
    query_b = np.asarray(inputs["query_b"], np.float32)
    annot_w = np.asarray(inputs["annot_w"], np.float32)
    annot_b = np.asarray(inputs["annot_b"], np.float32)
    v_w = np.asarray(inputs["v_w"], np.float32)

    wihT = np.zeros((640, 3 * H), np.float32)
    wihT[:KX] = W_ih.T
    w1p = np.ascontiguousarray(
        wihT.reshape(5, 128, 3 * H).transpose(1, 0, 2).reshape(128, 5 * 3 * H)
    ).astype(bf)
    whhT = np.ascontiguousarray(W_hh.T)
    w2p = np.ascontiguousarray(
        whhT.reshape(4, 128, 3 * H).transpose(1, 0, 2).reshape(128, 4 * 3 * H)
    ).astype(bf)
    qwT = np.ascontiguousarray(query_w.T)
    w3p = np.ascontiguousarray(
        qwT.reshape(4, 128, H).transpose(1, 0, 2).reshape(128, 4 * H)
    ).astype(bf)
    awT = np.ascontiguousarray(annot_w.T)
    awp = np.ascontiguousarray(
        awT.reshape(4, 128, H).transpose(1, 0, 2).reshape(128, 4 * H)
    ).astype(bf)

    g = loc_lin_w @ loc_conv_w.reshape(LOC_DIM, 2 * LOC_K)  # (512, 62)
    gT = np.ascontiguousarray(g.T)
    ab_col = (query_b + annot_b + loc_lin_b).reshape(4, 128).T  # (128, 4)
    v_col = v_w.reshape(4, 128).T.astype(bf)

    av_pad = np.zeros((B, 2, T + 2 * LOC_PAD), np.float32)
    av_pad[:, :, LOC_PAD : LOC_PAD + T] = attention_vec
    im2col = np.empty((B, 2, LOC_K, T), np.float32)
    for k in range(LOC_K):
        im2col[:, :, k, :] = av_pad[:, :, k : k + T]
    im2col = im2col.reshape(B, 2 * LOC_K, T)

    x = np.concatenate([memory, context], axis=1)
    xpad = np.zeros((B, 640), np.float32)
    xpad[:, :KX] = x

    annT = annotations.transpose(0, 2, 1).astype(bf)  # (B, H, T)
    annp_all = np.ascontiguousarray(
        annT.reshape(B // BL, BL, 4, 128, T).transpose(0, 3, 1, 2, 4)
    )  # (ncores, 128, BL, 4, T)

    in_maps = []
    for c in range(NCORES):
        sl = slice(c * BL, (c + 1) * BL)

        sb0p = np.zeros((128, _SB0_COLS), bf)
        sb0p[0:1, _B_ONE : _B_ONE + 128] = bf(1.0)
        sb0p[:, _B_V : _B_V + 4] = v_col
        sb0p[0:62, _B_GT : _B_GT + 512] = gT.astype(bf)
        sb1p = np.zeros((128, _SB1_COLS), bf)
        sb1p[:, _B_XT : _B_XT + 40] = (
            xpad[sl].reshape(BL, 5, 128).transpose(2, 1, 0).reshape(128, 40)
        ).astype(bf)
        sb1p[:, _B_HT : _B_HT + 32] = (
            rnn_state[sl].reshape(BL, 4, 128).transpose(2, 1, 0).reshape(128, 32)
        ).astype(bf)
        sb1p[0:1, _B_ON8 : _B_ON8 + BL] = bf(1.0)
        sb1p[0:1, _B_BIH : _B_BIH + 1536] = b_ih.astype(bf)
        sb1p[0:1, _B_BHH : _B_BHH + 1536] = b_hh.astype(bf)

        sfp = np.zeros((128, _SF_COLS), np.float32)
        sfp[:, _AB0 : _AB0 + 4] = ab_col
        sfp[0:BL, _HN0 : _HN0 + 512] = rnn_state[sl]
        sfp[0:BL, _ID80 : _ID80 + 8] = np.eye(8)
        sfp[:, _IDF0 : _IDF0 + 128] = np.eye(128)

        sb0p[0:62, _B_IM2 : _B_IM2 + BL * T] = (
            im2col[sl].transpose(1, 0, 2).reshape(62, BL * T).astype(bf)
        )

        in_maps.append(
            dict(
                annp=annp_all[c],
                w1p=w1p,
                w2p=w2p,
                w3p=w3p,
                awp=awp,
                sfp=sfp,
                sb0p=sb0p,
                sb1p=sb1p,
            )
        )
    return in_maps


def run(inputs, trace=False, **kw):
    from concourse.bass_utils import run_bass_kernel_spmd

    nc = _get_nc()
    in_maps = _host_prep(inputs)
    res = run_bass_kernel_spmd(nc, in_maps, list(range(NCORES)), trace=trace, **kw)
    rnn = np.concatenate([res.results[c]["rnn_out"] for c in range(NCORES)], axis=0)
    ctx = np.concatenate([res.results[c]["ctx_out"] for c in range(NCORES)], axis=0)
    al = np.concatenate([res.results[c]["align_out"] for c in range(NCORES)], axis=0)
    return (rnn, ctx, al), res


def kernel(**inputs):
    (rnn, ctx, al), _ = run(inputs)
    return (rnn, ctx, al)
